# revision 4
# baseline (speedup 1.0000x reference)
"""BotRGCN forward on 8 TRN2 NeuronCores (Bass/Tile SPMD kernel), v3.

Strategy (self-contained; shapes hardcoded for nn_BotRGCN1):
  - Nodes sharded 8-way (6272/core, N padded 50000->50176); f16 on-chip.
  - Dense MLPs node-parallel, feature-major on-chip ([128 feat, nodes]).
  - RGCN layer: aggregate-then-transform.  Edge messages are gathered
    with dma_gather (f16 node features, 256B rows; CHUNK=512 tokens per
    gather keeps the 4-deep Pool exec queue streaming) and segment-summed
    on the TensorEngine via per-block weighted one-hot matmuls
    Ind[tok,dst] = (iota==dst_local), one single-scalar DVE is_equal per
    block (2x DVE rate), accumulated in PSUM per (region, tile-384, rel).
    Mean-normalization happens at the drain: acc = agg * rcp[dst], a
    per-column rcp row streamed from DRAM (ACT copy + DVE multiply).
    Transform = W_r^T @ acc on PE; all matmuls f16 (4x fp32 PE rate).
  - Boundary exchange: layer outputs split at local row 3072 into lo/hi
    halves; each half AllGathers as soon as it is ready so the collective
    overlaps compute.  Sources renumbered region-major (xf_a row =
    core*3072+row, xf_b row = core*3200+(row-3072)); int16-safe (<32768).
  - DVE runs a pure ind-build stream: region-a partials re-enter PSUM via
    an identity matmul, finish writes go through the Scalar engine, so
    the in-order DVE queue never blocks on PE.
  - Leaky ReLU = one scalar-engine Prelu (alpha=0.01) with fused bias.
  - Edge schedule = max over cores (SPMD: one program for all 8).
"""
import numpy as np
import ml_dtypes

N = 50000
M = 8
L = 6272            # nodes per core (N padded to 50176)
NPAD = M * L
D = 128
DDES = 768
R = 5
TW = 384            # dst tile width
NT = 17             # 16 full tiles + 1 of 128
RA = 3072           # region-a rows per core (tiles 0..7; MLP chunks 0..5)
RB = L - RA         # 3200 (tiles 8..16)
GA = M * RA         # rows in xf_a
GB = M * RB         # rows in xf_b
CHUNK = 4096        # tokens per dma_gather (amortize ~1us SWDGE fixed overhead)
MCH = 512           # MLP chunk width (13 chunks: 12x512 + 128)
NCH = 13
SLOPE = 0.01
NQUEUES = 4

_LAST = {}          # exec stats for test harness


def _tile_w(t):
    return min(TW, L - t * TW)


def _mch_w(c):
    return min(MCH, L - c * MCH)


def _prep_edges(edge_index, edge_type):
    """Per-core token streams + shared (max-over-cores) block schedule."""
    src = np.asarray(edge_index[0], dtype=np.int64)
    dst = np.asarray(edge_index[1], dtype=np.int64)
    et = np.asarray(edge_type, dtype=np.int64)

    core = dst // L
    dloc = dst % L

    # group tokens per core: key = (region, tile, rel)
    per_core_groups = []
    rcp_rep_all = []    # per core: [NT*R, 128, TW] f16 rcp rows for the drain
    for m in range(M):
        sel = np.nonzero(core == m)[0]
        s, dl, r = src[sel], dloc[sel], et[sel]
        cnt = np.zeros((R, L), np.float32)
        np.add.at(cnt, (r, dl), 1.0)
        rcp = 1.0 / np.maximum(cnt, 1.0)
        rows = np.ones((NT * R, TW), np.float16)
        for tt in range(NT):
            w = _tile_w(tt)
            for rr in range(R):
                rows[tt * R + rr, :w] = rcp[rr, tt * TW:tt * TW + w]
        rcp_rep_all.append(
            np.broadcast_to(rows[:, None, :], (NT * R, 128, TW)).copy())
        sc, sl = s // L, s % L
        reg = (sl >= RA).astype(np.int64)
        ridx = np.where(reg == 0, sc * RA + sl, sc * RB + (sl - RA))
        t = dl // TW
        key = (reg * NT + t) * R + r
        order = np.argsort(key, kind="stable")
        ridx, dl, r, key = ridx[order], dl[order], r[order], key[order]
        t = t[order]
        groups = {}
        bounds = np.searchsorted(key, np.arange(2 * NT * R + 1))
        for gk in range(2 * NT * R):
            a, b = bounds[gk], bounds[gk + 1]
            greg, gt, gr = gk // (NT * R), (gk // R) % NT, gk % R
            gidx = ridx[a:b].astype(np.int16)
            gdst = (dl[a:b] - gt * TW).astype(np.float32)
            grcp = rcp[r[a:b], dl[a:b]].astype(np.float32)
            # sort by dst within the group so each 128-token block spans a
            # narrow contiguous dst range (narrow ind build + agg matmul)
            o2 = np.argsort(gdst, kind="stable")
            groups[(greg, gt, gr)] = (gidx[o2], gdst[o2], grcp[o2])
        per_core_groups.append(groups)

    # shared schedule: blocks per group = max over cores (>=1)
    nblk = {}
    for greg in range(2):
        for gt in range(NT):
            for gr in range(R):
                mx = max(len(per_core_groups[m][(greg, gt, gr)][0])
                         for m in range(M))
                nblk[(greg, gt, gr)] = max(1, -(-mx // 128))

    # pad each region to a CHUNK multiple (prepare_only gathers must all be
    # full CHUNK; partial preps derail the SWDGE ring) by extending the last
    # group with pad blocks (gdst=-1 -> ind==0 -> harmless)
    for greg in range(2):
        tot = 128 * sum(nblk[(greg, gt, gr)]
                        for gt in range(NT) for gr in range(R))
        deficit = (-tot) % CHUNK
        nblk[(greg, NT - 1, R - 1)] += deficit // 128

    # build padded per-core streams in fixed (region, tile, rel) order.
    # pad tokens use gdst=1000 (matches no iota value 0..TW-1, sorts last).
    TTOT = 128 * sum(nblk.values())
    NBLK = TTOT // 128
    gidx_all = np.zeros((M, TTOT), np.int16)
    gdst_all = np.full((M, TTOT), 1000.0, np.float32)
    grcp_all = np.zeros((M, TTOT), np.float32)
    pos = 0
    sched = []   # per block: (region, tile, rel, first, last, lo, span)
    reg_tok = [0, 0]
    for greg in range(2):
        for gt in range(NT):
            for gr in range(R):
                nb = nblk[(greg, gt, gr)]
                for m in range(M):
                    gi, gd, gc = per_core_groups[m][(greg, gt, gr)]
                    n = len(gi)
                    gidx_all[m, pos:pos + n] = gi
                    gdst_all[m, pos:pos + n] = gd
                    grcp_all[m, pos:pos + n] = gc
                for j in range(nb):
                    blk = gdst_all[:, pos + j * 128:pos + (j + 1) * 128]
                    real = blk[blk < 999.0]
                    if len(real):
                        lo, hi = int(real.min()), int(real.max()) + 1
                    else:
                        lo, hi = 0, 1
                    sched.append((greg, gt, gr, j == 0, j == nb - 1,
                                  lo, hi - lo))
                pos += nb * 128
                reg_tok[greg] += nb * 128
    assert pos == TTOT

    # gather chunks: per region, cut every CHUNK tokens
    chunks = []  # (region, tok_start, ntok)
    off = 0
    for greg in range(2):
        th = reg_tok[greg]
        s0 = 0
        while s0 < th:
            n = min(CHUNK, th - s0)
            chunks.append((greg, off + s0, n))
            s0 += n
        off += th

    # wrap layouts for SBUF
    gidx_w = np.tile(
        gidx_all.reshape(M, TTOT // 16, 16).transpose(0, 2, 1), (1, 8, 1)
    ).copy()                                            # [M, 128, TTOT//16]
    gdst_w = gdst_all.reshape(M, NBLK, 128).transpose(0, 2, 1).copy()  # [M,128,NBLK]
    grcp_w = grcp_all.reshape(M, NBLK, 128).transpose(0, 2, 1).copy()
    return gidx_w, gdst_w, grcp_w, sched, chunks, TTOT, NBLK, rcp_rep_all


def _build(sched, chunks, TTOT, NBLK):
    from concourse import bacc, tile, mybir

    nc = bacc.Bacc("TRN2", target_bir_lowering=False, debug=False,
                   num_devices=M, num_swdge_queues=NQUEUES)
    f32, i16 = mybir.dt.float32, mybir.dt.int16
    f16 = mybir.dt.float16
    Alu = mybir.AluOpType
    Act = mybir.ActivationFunctionType

    desT_d = nc.dram_tensor("desT", [DDES, L], f16, kind="ExternalInput")
    gidx_d = nc.dram_tensor("gidx", [128, TTOT // 16], i16, kind="ExternalInput")
    gdst_d = nc.dram_tensor("gdst", [128, NBLK], f32, kind="ExternalInput")
    grcp_d = nc.dram_tensor("grcp", [128, NBLK], f32, kind="ExternalInput")
    rcpr_d = nc.dram_tensor("rcpr", [NT * R, 128, TW], f16, kind="ExternalInput")
    wdes_d = nc.dram_tensor("wdes", [DDES, D], f16, kind="ExternalInput")
    win_d = nc.dram_tensor("win", [D, D], f16, kind="ExternalInput")
    wroot_d = nc.dram_tensor("wroot", [D, D], f16, kind="ExternalInput")
    wrel_d = nc.dram_tensor("wrel", [R, D, D], f16, kind="ExternalInput")
    wout1_d = nc.dram_tensor("wout1", [D, D], f16, kind="ExternalInput")
    wout2_d = nc.dram_tensor("wout2", [D, 2], f16, kind="ExternalInput")
    bias_d = nc.dram_tensor("bias", [D, 4], f32, kind="ExternalInput")  # des,in,rgcn,out1
    bout2_d = nc.dram_tensor("bout2", [2, 1], f32, kind="ExternalInput")
    out_d = nc.dram_tensor("out", [2, L], f32, kind="ExternalOutput")

    y_lo = [nc.dram_tensor(f"y{i}_lo", [RA, D], f16) for i in range(2)]
    y_hi = [nc.dram_tensor(f"y{i}_hi", [RB, D], f16) for i in range(2)]
    xf_a = [nc.dram_tensor(f"xf{i}_a", [GA, D], f16, addr_space="Shared")
            for i in range(2)]
    xf_b = [nc.dram_tensor(f"xf{i}_b", [GB, D], f16, addr_space="Shared")
            for i in range(2)]

    iota = nc.inline_tensor(
        np.broadcast_to(np.arange(TW, dtype=np.float16), (128, TW)).copy(), "iota")
    ident = nc.inline_tensor(np.eye(128, dtype=np.float16), "ident")

    with tile.TileContext(nc) as tc:
        with (
            tc.tile_pool(name="cst", bufs=1) as cst,
            tc.tile_pool(name="big", bufs=2) as big,
            tc.tile_pool(name="wk", bufs=4) as wk,
            tc.tile_pool(name="ps", bufs=1, space="PSUM") as psp,
        ):
            # ---- constants to SBUF ----
            iota_sb = cst.tile([128, TW], f16)
            nc.sync.dma_start(out=iota_sb[:], in_=iota[:])
            ident_sb = cst.tile([128, 128], f16)
            nc.sync.dma_start(out=ident_sb[:], in_=ident[:])
            gidx_sb = cst.tile([128, TTOT // 16], i16)
            nc.sync.dma_start(out=gidx_sb[:], in_=gidx_d[:])
            gdst_sb = cst.tile([128, NBLK], f32)
            nc.sync.dma_start(out=gdst_sb[:], in_=gdst_d[:])
            grcp_sb = cst.tile([128, NBLK], f32)
            nc.sync.dma_start(out=grcp_sb[:], in_=grcp_d[:])
            wdes_sb = cst.tile([128, 6, D], f16)
            for k in range(6):
                nc.sync.dma_start(out=wdes_sb[:, k, :], in_=wdes_d[k * 128:(k + 1) * 128, :])
            win_sb = cst.tile([128, D], f16)
            nc.sync.dma_start(out=win_sb[:], in_=win_d[:])
            wroot_sb = cst.tile([128, D], f16)
            nc.sync.dma_start(out=wroot_sb[:], in_=wroot_d[:])
            wrel_sb = cst.tile([128, R, D], f16)
            for r in range(R):
                nc.sync.dma_start(out=wrel_sb[:, r, :], in_=wrel_d[r])
            wout1_sb = cst.tile([128, D], f16)
            nc.sync.dma_start(out=wout1_sb[:], in_=wout1_d[:])
            wout2_sb = cst.tile([128, 2], f16)
            nc.sync.dma_start(out=wout2_sb[:], in_=wout2_d[:])
            bias_sb = cst.tile([128, 4], f32)
            nc.sync.dma_start(out=bias_sb[:], in_=bias_d[:])
            bout2_sb = cst.tile([2, 1], f32)
            nc.sync.dma_start(out=bout2_sb[:], in_=bout2_d[:])

            def all_gather(src_d, dst_d):
                nc.gpsimd.collective_compute(
                    "AllGather", mybir.AluOpType.bypass,
                    replica_groups=[list(range(M))],
                    ins=[src_d[:]], outs=[dst_d[:]])

            def transpose_store(src_f16_ap, row0, w, ylo_d, yhi_d):
                """feature-major f16 [128, w] -> node-major rows of y lo/hi."""
                for b in range(-(-w // 128)):
                    bw = min(128, w - b * 128)
                    trp = psp.tile([128, 128], f16, tag="tr")
                    nc.tensor.transpose(
                        trp[:bw, :], src_f16_ap[:, b * 128:b * 128 + bw], ident_sb[:])
                    ynm = wk.tile([128, D], f16, tag="ynm")
                    nc.scalar.activation(ynm[:bw, :], trp[:bw, :], Act.Copy)
                    r0 = row0 + b * 128
                    if r0 < RA:
                        nc.sync.dma_start(out=ylo_d[r0:r0 + bw, :], in_=ynm[:bw, :])
                    else:
                        nc.sync.dma_start(out=yhi_d[r0 - RA:r0 - RA + bw, :],
                                          in_=ynm[:bw, :])

            # ================= MLP =================
            x1T = big.tile([128, L], f16, tag="bigT")
            for c in range(NCH):
                w = _mch_w(c)
                ps = psp.tile([128, MCH], f32, tag="out", bufs=2)
                for k in range(6):
                    dt = wk.tile([128, MCH], f16, tag="des")
                    nc.sync.dma_start(
                        out=dt[:, :w],
                        in_=desT_d[k * 128:(k + 1) * 128, c * MCH:c * MCH + w])
                    nc.tensor.matmul(ps[:, :w], wdes_sb[:, k, :], dt[:, :w],
                                     start=(k == 0), stop=(k == 5))
                x0c = wk.tile([128, MCH], f16, tag="x0c")
                nc.scalar.activation(x0c[:, :w], ps[:, :w], Act.Prelu,
                                     bias=bias_sb[:, 0:1], alpha=SLOPE)
                ps2 = psp.tile([128, MCH], f32, tag="out", bufs=2)
                nc.tensor.matmul(ps2[:, :w], win_sb[:], x0c[:, :w],
                                 start=True, stop=True)
                nc.scalar.activation(x1T[:, c * MCH:c * MCH + w], ps2[:, :w],
                                     Act.Prelu, bias=bias_sb[:, 1:2], alpha=SLOPE)
                transpose_store(x1T[:, c * MCH:c * MCH + w], c * MCH, w,
                                y_lo[0], y_hi[0])
                if c == RA // MCH - 1:          # rows [0, RA) stored
                    all_gather(y_lo[0], xf_a[0])
            all_gather(y_hi[0], xf_b[0])

            # ================= RGCN layers =================
            # queue must track Tile's global DMASW lane rotation: gathers are
            # the ONLY Pool-engine DMA insts, so a single global counter keeps
            # lane%4 == queue for every gather.
            qctr = [0]

            def rgcn_layer(xfa, xfb, x_curT, ylo_d, yhi_d, nxt):
                """nxt = (xf_a', xf_b') to AllGather into, or None if last."""
                yT = big.tile([128, L], f16, tag="bigT")
                xf_base = [xfa, xfb]
                agg = {}       # r -> psum tile for current (region, t)
                accTs = {}     # r -> drained SBUF acc for current (region, t)

                def finish_tile(reg, t):
                    w = _tile_w(t)
                    sl = yT[:, t * TW:t * TW + w]
                    ops = psp.tile([128, TW], f32, tag="out", bufs=2)
                    if reg == 0:
                        nc.tensor.matmul(ops[:, :w], wroot_sb[:],
                                         x_curT[:, t * TW:t * TW + w],
                                         start=True, stop=False)
                    else:
                        # pull region-a partial back into PSUM via identity
                        # matmul so DVE stays a pure ind-build stream
                        nc.tensor.matmul(ops[:, :w], ident_sb[:], sl,
                                         start=True, stop=False)
                    for ri in range(R):
                        nc.tensor.matmul(ops[:, :w], wrel_sb[:, ri, :],
                                         accTs[ri][:, :w],
                                         start=False,
                                         stop=(ri == R - 1))
                    if reg == 0:
                        nc.scalar.activation(sl, ops[:, :w], Act.Identity,
                                             bias=bias_sb[:, 2:3])
                    else:
                        nc.scalar.activation(sl, ops[:, :w], Act.Copy)
                        if nxt is not None:
                            transpose_store(sl, t * TW, w, ylo_d, yhi_d)
                            if t == RA // TW - 1:
                                all_gather(ylo_d, nxt[0])
                            elif t == NT - 1:
                                all_gather(yhi_d, nxt[1])
                        else:
                            # last layer: fuse the output MLP per finished
                            # tile so it overlaps the remaining aggregation
                            ps_o = psp.tile([128, TW], f32, tag="out", bufs=2)
                            nc.tensor.matmul(ps_o[:, :w], wout1_sb[:], sl,
                                             start=True, stop=True)
                            z1 = wk.tile([128, TW], f16, tag="x0c")
                            nc.scalar.activation(z1[:, :w], ps_o[:, :w],
                                                 Act.Prelu,
                                                 bias=bias_sb[:, 3:4],
                                                 alpha=SLOPE)
                            ps2 = psp.tile([2, TW], f32, tag="out2")
                            nc.tensor.matmul(ps2[:, :w], wout2_sb[:],
                                             z1[:, :w], start=True, stop=True)
                            nc.scalar.activation(outT[:, t * TW:t * TW + w],
                                                 ps2[:, :w], Act.Identity,
                                                 bias=bout2_sb[:, 0:1])
                    accTs.clear()

                blk_i = 0
                cur = None  # (region, t)
                for (reg, s0, ntok) in chunks:
                    nb = ntok // 128
                    g = wk.tile([128, CHUNK // 128, D], f16, tag="g", bufs=4)
                    q = qctr[0] % NQUEUES
                    nc.gpsimd.dma_gather(
                        out_ap=g[:, :nb, :],
                        in_ap=xf_base[reg][:],
                        idxs_ap=gidx_sb[:, s0 // 16:(s0 + ntok) // 16],
                        num_idxs=ntok,
                        num_idxs_reg=ntok,
                        elem_size=D,
                        single_packet=False,
                        queue_num=q,
                    )
                    qctr[0] += 1
                    for j in range(nb):
                        breg, bt, br, first, last, lo, span = sched[blk_i]
                        assert breg == reg
                        if cur is None:
                            cur = (breg, bt)
                        elif cur != (breg, bt):
                            finish_tile(*cur)
                            cur = (breg, bt)
                        w = _tile_w(bt)
                        col = s0 // 128 + j
                        ind = wk.tile([128, TW], f16, tag="ind", bufs=56)
                        nc.vector.tensor_scalar(
                            out=ind[:, :w], in0=iota_sb[:, :w],
                            scalar1=gdst_sb[:, col:col + 1],
                            scalar2=None, op0=Alu.is_equal)
                        if first:
                            agg[br] = psp.tile([128, TW], f32, tag="agg",
                                               name=f"agg{br}", bufs=4)
                        nc.tensor.matmul(agg[br][:, :w], g[:, j, :],
                                         ind[:, :w], start=first, stop=last)
                        if last:
                            # drain + mean-normalize: acc = agg * rcp[dst]
                            # (per-column rcp row, replicated across
                            # partitions, streamed from DRAM)
                            rt = wk.tile([128, TW], f16, tag="rt", bufs=8)
                            nc.sync.dma_start(
                                out=rt[:, :w],
                                in_=rcpr_d[bt * R + br, :, :w])
                            acc = wk.tile([128, TW], f16, tag="accT", bufs=12)
                            nc.scalar.activation(acc[:, :w], agg[br][:, :w],
                                                 Act.Copy)
                            accM = wk.tile([128, TW], f16, tag="accM", bufs=12)
                            nc.vector.tensor_tensor(
                                out=accM[:, :w], in0=acc[:, :w],
                                in1=rt[:, :w], op=Alu.mult)
                            accTs[br] = accM
                        blk_i += 1
                finish_tile(*cur)
                assert blk_i == len(sched)
                return yT

            outT = big.tile([2, L], f32, tag="outT")
            y1T = rgcn_layer(xf_a[0], xf_b[0], x1T, y_lo[1], y_hi[1],
                             nxt=(xf_a[1], xf_b[1]))
            rgcn_layer(xf_a[1], xf_b[1], y1T, None, None, nxt=None)
            nc.sync.dma_start(out=out_d[:], in_=outT[:])

    nc.compile()
    return nc


def kernel(des, tweet, num_prop, cat_prop, edge_index, edge_type,
           W_des, b_des, W_in, b_in, W_rel, W_root, b_rgcn,
           W_out1, b_out1, W_out2, b_out2):
    import time
    from concourse.bass_utils import run_bass_kernel_spmd

    des = np.asarray(des, np.float32)
    gidx_w, gdst_w, grcp_w, sched, chunks, TTOT, NBLK, rcp_rep = _prep_edges(
        np.asarray(edge_index), np.asarray(edge_type))

    t0 = time.time()
    nc = _build(sched, chunks, TTOT, NBLK)
    t1 = time.time()

    des_pad = np.zeros((NPAD, DDES), np.float16)
    des_pad[:N] = des.astype(np.float16)
    bias = np.stack([np.asarray(b_des, np.float32),
                     np.asarray(b_in, np.float32),
                     np.asarray(b_rgcn, np.float32),
                     np.asarray(b_out1, np.float32)], axis=1)  # [128,4]
    common = {
        "wdes": np.asarray(W_des, np.float16),
        "win": np.asarray(W_in, np.float16),
        "wroot": np.asarray(W_root, np.float16),
        "wrel": np.asarray(W_rel, np.float16),
        "wout1": np.asarray(W_out1, np.float16),
        "wout2": np.asarray(W_out2, np.float16),
        "bias": bias,
        "bout2": np.asarray(b_out2, np.float32).reshape(2, 1),
    }
    in_maps = []
    for m in range(M):
        in_maps.append({
            "desT": np.ascontiguousarray(des_pad[m * L:(m + 1) * L].T),
            "gidx": gidx_w[m], "gdst": gdst_w[m], "grcp": grcp_w[m],
            "rcpr": rcp_rep[m],
            **common,
        })

    trace = bool(_LAST.get("trace"))
    res = run_bass_kernel_spmd(nc, in_maps, list(range(M)), trace=trace)
    t2 = time.time()
    _LAST["build_s"] = t1 - t0
    _LAST["run_s"] = t2 - t1
    _LAST["exec_ns"] = res.exec_time_ns
    _LAST["ttot"] = TTOT

    out = np.concatenate([res.results[m]["out"].T for m in range(M)], axis=0)
    return np.ascontiguousarray(out[:N])



# revision 5
# speedup vs baseline: 2.0937x; 2.0937x over previous
"""BotRGCN forward on 8 TRN2 NeuronCores (Bass/Tile SPMD kernel), v4.

Strategy (self-contained; shapes hardcoded for nn_BotRGCN1):
  - Nodes sharded 8-way (6272/core, N padded 50000->50176); f16 on-chip.
  - Dense MLPs node-parallel, feature-major on-chip ([128 feat, nodes]).
  - RGCN layer: aggregate-then-transform.  Edge messages gathered with
    dma_gather (f16 node rows, 256B; CHUNK=512 tokens/gather) and
    segment-summed on the TensorEngine via per-block one-hot matmuls.
    v4: the one-hot matrices are HOST-PRECOMPUTED narrow strips
    ([128 tok, span] f16, span = dst range of the sorted block, with the
    per-edge mean-normalization rcp FOLDED INTO the strip values).  They
    stream from DRAM once per layer; the DVE is_equal build and the
    per-group rcp drain multiply of v3 are gone.  Aggregation matmuls run
    narrow ([lo, lo+span) of the 384-wide dst tile).  Each (reg,tile,rel)
    group's PSUM is zeroed by a DVE memset first (narrow first matmul
    only clears has_written for the bank; content outside its N range
    must be real zeros for the transform read).
  - Transform = W_r^T @ acc on PE; all matmuls f16.
  - Boundary exchange: layer outputs split at local row 3072 into lo/hi
    halves; each half AllGathers as soon as it is ready so the collective
    overlaps compute.  Sources renumbered region-major (int16-safe).
  - Leaky ReLU = one scalar-engine Prelu (alpha=0.01) with fused bias.
  - Edge schedule = max over cores (SPMD: one program for all 8).
"""
import numpy as np
import ml_dtypes

N = 50000
M = 8
L = 6272            # nodes per core (N padded to 50176)
NPAD = M * L
D = 128
DDES = 768
R = 5
TW = 384            # dst tile width
NT = 17             # 16 full tiles + 1 of 128
RA = 3072           # region-a rows per core (tiles 0..7; MLP chunks 0..5)
RB = L - RA         # 3200 (tiles 8..16)
GA = M * RA         # rows in xf_a
GB = M * RB         # rows in xf_b
CHUNK = 512         # tokens per dma_gather
MCH = 512           # MLP chunk width (13 chunks: 12x512 + 128)
NCH = 13
SLOPE = 0.01
NQUEUES = 4

_LAST = {}          # exec stats for test harness


def _tile_w(t):
    return min(TW, L - t * TW)


def _mch_w(c):
    return min(MCH, L - c * MCH)


def _prep_edges(edge_index, edge_type):
    """Per-core token streams + shared (max-over-cores) block schedule.

    Returns host-precomputed one-hot strips: for each 128-token block the
    [128, span] f16 matrix S with S[tok, dst - lo] = rcp(tok), used as the
    moving operand of the aggregation matmul."""
    src = np.asarray(edge_index[0], dtype=np.int64)
    dst = np.asarray(edge_index[1], dtype=np.int64)
    et = np.asarray(edge_type, dtype=np.int64)

    core = dst // L
    dloc = dst % L

    # group tokens per core: key = (region, tile, rel)
    per_core_groups = []
    for m in range(M):
        sel = np.nonzero(core == m)[0]
        s, dl, r = src[sel], dloc[sel], et[sel]
        cnt = np.zeros((R, L), np.float32)
        np.add.at(cnt, (r, dl), 1.0)
        rcp = 1.0 / np.maximum(cnt, 1.0)
        sc, sl = s // L, s % L
        reg = (sl >= RA).astype(np.int64)
        ridx = np.where(reg == 0, sc * RA + sl, sc * RB + (sl - RA))
        t = dl // TW
        key = (reg * NT + t) * R + r
        order = np.argsort(key, kind="stable")
        ridx, dl, r, key = ridx[order], dl[order], r[order], key[order]
        groups = {}
        bounds = np.searchsorted(key, np.arange(2 * NT * R + 1))
        for gk in range(2 * NT * R):
            a, b = bounds[gk], bounds[gk + 1]
            greg, gt, gr = gk // (NT * R), (gk // R) % NT, gk % R
            gidx = ridx[a:b].astype(np.int16)
            gdst = (dl[a:b] - gt * TW).astype(np.float32)
            grcp = rcp[r[a:b], dl[a:b]].astype(np.float32)
            # sort by dst within the group so each 128-token block spans a
            # narrow contiguous dst range (narrow strip + agg matmul)
            o2 = np.argsort(gdst, kind="stable")
            groups[(greg, gt, gr)] = (gidx[o2], gdst[o2], grcp[o2])
        per_core_groups.append(groups)

    # shared schedule: blocks per group = max over cores (>=1)
    nblk = {}
    for greg in range(2):
        for gt in range(NT):
            for gr in range(R):
                mx = max(len(per_core_groups[m][(greg, gt, gr)][0])
                         for m in range(M))
                nblk[(greg, gt, gr)] = max(1, -(-mx // 128))

    # pad each region to a CHUNK multiple by extending the last group with
    # pad blocks (gdst=1000 -> zero strip col -> harmless)
    for greg in range(2):
        tot = 128 * sum(nblk[(greg, gt, gr)]
                        for gt in range(NT) for gr in range(R))
        deficit = (-tot) % CHUNK
        nblk[(greg, NT - 1, R - 1)] += deficit // 128

    # build padded per-core streams in fixed (region, tile, rel) order.
    # pad tokens use gdst=1000 (sorts last, strip value 0).
    TTOT = 128 * sum(nblk.values())
    NBLK = TTOT // 128
    gidx_all = np.zeros((M, TTOT), np.int16)
    gdst_all = np.full((M, TTOT), 1000.0, np.float32)
    grcp_all = np.zeros((M, TTOT), np.float32)
    pos = 0
    sched = []   # per block: (region, tile, rel, first, last, lo, span, soff)
    reg_tok = [0, 0]
    soff = 0
    for greg in range(2):
        for gt in range(NT):
            for gr in range(R):
                nb = nblk[(greg, gt, gr)]
                for m in range(M):
                    gi, gd, gc = per_core_groups[m][(greg, gt, gr)]
                    n = len(gi)
                    gidx_all[m, pos:pos + n] = gi
                    gdst_all[m, pos:pos + n] = gd
                    grcp_all[m, pos:pos + n] = gc
                for j in range(nb):
                    blk = gdst_all[:, pos + j * 128:pos + (j + 1) * 128]
                    real = blk[blk < 999.0]
                    if len(real):
                        lo, hi = int(real.min()), int(real.max()) + 1
                    else:
                        lo, hi = 0, 1
                    sched.append((greg, gt, gr, j == 0, j == nb - 1,
                                  lo, hi - lo, soff))
                    soff += hi - lo
                pos += nb * 128
                reg_tok[greg] += nb * 128
    assert pos == TTOT
    TOTCOL = soff

    # per-core strips [128, TOTCOL] f16
    strips = np.zeros((M, 128, TOTCOL), np.float16)
    rows = np.arange(128)
    for bi, (greg, gt, gr, first, last, lo, span, so) in enumerate(sched):
        tok0 = bi * 128
        for m in range(M):
            gd = gdst_all[m, tok0:tok0 + 128]
            gc = grcp_all[m, tok0:tok0 + 128]
            valid = gd < 999.0
            cols = (gd - lo).astype(np.int64)
            cols[~valid] = 0
            vals = np.where(valid, gc, 0.0).astype(np.float16)
            blkm = np.zeros((128, span), np.float16)
            blkm[rows, cols] = vals
            # a pad token with col 0 writes 0.0 there; real token at col 0 of
            # the same row cannot co-exist (one row = one token)
            strips[m, :, so:so + span] = blkm

    # gather chunks: per region, cut every CHUNK tokens; strip col ranges
    chunks = []  # (region, tok_start, ntok, strip_off, strip_cols)
    off = 0
    blk_i = 0
    for greg in range(2):
        th = reg_tok[greg]
        s0 = 0
        while s0 < th:
            n = min(CHUNK, th - s0)
            nb = n // 128
            so = sched[blk_i][7]
            ccols = sum(sched[blk_i + k][6] for k in range(nb))
            chunks.append((greg, off + s0, n, so, ccols))
            blk_i += nb
            s0 += n
        off += th
    SMAXC = max(c[4] for c in chunks)

    # wrap gidx for SBUF
    gidx_w = np.tile(
        gidx_all.reshape(M, TTOT // 16, 16).transpose(0, 2, 1), (1, 8, 1)
    ).copy()                                            # [M, 128, TTOT//16]
    return gidx_w, strips, sched, chunks, TTOT, TOTCOL, SMAXC


def _build(sched, chunks, TTOT, TOTCOL, SMAXC):
    from concourse import bacc, tile, mybir

    nc = bacc.Bacc("TRN2", target_bir_lowering=False, debug=False,
                   num_devices=M, num_swdge_queues=NQUEUES)
    f32, i16 = mybir.dt.float32, mybir.dt.int16
    f16 = mybir.dt.float16
    Act = mybir.ActivationFunctionType

    desT_d = nc.dram_tensor("desT", [DDES, L], f16, kind="ExternalInput")
    gidx_d = nc.dram_tensor("gidx", [128, TTOT // 16], i16, kind="ExternalInput")
    strips_d = nc.dram_tensor("strips", [128, TOTCOL], f16, kind="ExternalInput")
    wdes_d = nc.dram_tensor("wdes", [DDES, D], f16, kind="ExternalInput")
    win_d = nc.dram_tensor("win", [D, D], f16, kind="ExternalInput")
    wroot_d = nc.dram_tensor("wroot", [D, D], f16, kind="ExternalInput")
    wrel_d = nc.dram_tensor("wrel", [R, D, D], f16, kind="ExternalInput")
    wout1_d = nc.dram_tensor("wout1", [D, D], f16, kind="ExternalInput")
    wout2_d = nc.dram_tensor("wout2", [D, 2], f16, kind="ExternalInput")
    bias_d = nc.dram_tensor("bias", [D, 4], f32, kind="ExternalInput")  # des,in,rgcn,out1
    bout2_d = nc.dram_tensor("bout2", [2, 1], f32, kind="ExternalInput")
    out_d = nc.dram_tensor("out", [2, L], f32, kind="ExternalOutput")

    y_lo = [nc.dram_tensor(f"y{i}_lo", [RA, D], f16) for i in range(2)]
    y_hi = [nc.dram_tensor(f"y{i}_hi", [RB, D], f16) for i in range(2)]
    xf_a = [nc.dram_tensor(f"xf{i}_a", [GA, D], f16, addr_space="Shared")
            for i in range(2)]
    xf_b = [nc.dram_tensor(f"xf{i}_b", [GB, D], f16, addr_space="Shared")
            for i in range(2)]

    ident = nc.inline_tensor(np.eye(128, dtype=np.float16), "ident")

    with tile.TileContext(nc) as tc:
        with (
            tc.tile_pool(name="cst", bufs=1) as cst,
            tc.tile_pool(name="big", bufs=2) as big,
            tc.tile_pool(name="wk", bufs=4) as wk,
            tc.tile_pool(name="ps", bufs=1, space="PSUM") as psp,
        ):
            # ---- constants to SBUF ----
            ident_sb = cst.tile([128, 128], f16)
            nc.sync.dma_start(out=ident_sb[:], in_=ident[:])
            gidx_sb = cst.tile([128, TTOT // 16], i16)
            nc.sync.dma_start(out=gidx_sb[:], in_=gidx_d[:])
            wdes_sb = cst.tile([128, 6, D], f16)
            for k in range(6):
                nc.sync.dma_start(out=wdes_sb[:, k, :], in_=wdes_d[k * 128:(k + 1) * 128, :])
            win_sb = cst.tile([128, D], f16)
            nc.sync.dma_start(out=win_sb[:], in_=win_d[:])
            wroot_sb = cst.tile([128, D], f16)
            nc.sync.dma_start(out=wroot_sb[:], in_=wroot_d[:])
            wrel_sb = cst.tile([128, R, D], f16)
            for r in range(R):
                nc.sync.dma_start(out=wrel_sb[:, r, :], in_=wrel_d[r])
            wout1_sb = cst.tile([128, D], f16)
            nc.sync.dma_start(out=wout1_sb[:], in_=wout1_d[:])
            wout2_sb = cst.tile([128, 2], f16)
            nc.sync.dma_start(out=wout2_sb[:], in_=wout2_d[:])
            bias_sb = cst.tile([128, 4], f32)
            nc.sync.dma_start(out=bias_sb[:], in_=bias_d[:])
            bout2_sb = cst.tile([2, 1], f32)
            nc.sync.dma_start(out=bout2_sb[:], in_=bout2_d[:])

            def all_gather(src_d, dst_d):
                nc.gpsimd.collective_compute(
                    "AllGather", mybir.AluOpType.bypass,
                    replica_groups=[list(range(M))],
                    ins=[src_d[:]], outs=[dst_d[:]])

            def transpose_store(src_f16_ap, row0, w, ylo_d, yhi_d):
                """feature-major f16 [128, w] -> node-major rows of y lo/hi."""
                for b in range(-(-w // 128)):
                    bw = min(128, w - b * 128)
                    trp = psp.tile([128, 128], f16, tag="tr")
                    nc.tensor.transpose(
                        trp[:bw, :], src_f16_ap[:, b * 128:b * 128 + bw], ident_sb[:])
                    ynm = wk.tile([128, D], f16, tag="ynm")
                    nc.scalar.activation(ynm[:bw, :], trp[:bw, :], Act.Copy)
                    r0 = row0 + b * 128
                    if r0 < RA:
                        nc.sync.dma_start(out=ylo_d[r0:r0 + bw, :], in_=ynm[:bw, :])
                    else:
                        nc.sync.dma_start(out=yhi_d[r0 - RA:r0 - RA + bw, :],
                                          in_=ynm[:bw, :])

            # ================= MLP =================
            x1T = big.tile([128, L], f16, tag="bigT")
            for c in range(NCH):
                w = _mch_w(c)
                ps = psp.tile([128, MCH], f32, tag="out", bufs=2)
                for k in range(6):
                    dt = wk.tile([128, MCH], f16, tag="des")
                    nc.sync.dma_start(
                        out=dt[:, :w],
                        in_=desT_d[k * 128:(k + 1) * 128, c * MCH:c * MCH + w])
                    nc.tensor.matmul(ps[:, :w], wdes_sb[:, k, :], dt[:, :w],
                                     start=(k == 0), stop=(k == 5))
                x0c = wk.tile([128, MCH], f16, tag="x0c")
                nc.scalar.activation(x0c[:, :w], ps[:, :w], Act.Prelu,
                                     bias=bias_sb[:, 0:1], alpha=SLOPE)
                ps2 = psp.tile([128, MCH], f32, tag="out", bufs=2)
                nc.tensor.matmul(ps2[:, :w], win_sb[:], x0c[:, :w],
                                 start=True, stop=True)
                nc.scalar.activation(x1T[:, c * MCH:c * MCH + w], ps2[:, :w],
                                     Act.Prelu, bias=bias_sb[:, 1:2], alpha=SLOPE)
                transpose_store(x1T[:, c * MCH:c * MCH + w], c * MCH, w,
                                y_lo[0], y_hi[0])
                if c == RA // MCH - 1:          # rows [0, RA) stored
                    all_gather(y_lo[0], xf_a[0])
            all_gather(y_hi[0], xf_b[0])

            # ================= RGCN layers =================
            qctr = [0]

            def rgcn_layer(xfa, xfb, x_curT, ylo_d, yhi_d, nxt):
                """nxt = (xf_a', xf_b') to AllGather into, or None if last."""
                yT = big.tile([128, L], f16, tag="bigT")
                xf_base = [xfa, xfb]
                agg = {}       # r -> psum tile for current (region, t)
                accTs = {}     # r -> drained SBUF acc for current (region, t)

                def finish_tile(reg, t):
                    w = _tile_w(t)
                    sl = yT[:, t * TW:t * TW + w]
                    ops = psp.tile([128, TW], f32, tag="out", bufs=2)
                    if reg == 0:
                        nc.tensor.matmul(ops[:, :w], wroot_sb[:],
                                         x_curT[:, t * TW:t * TW + w],
                                         start=True, stop=False)
                    else:
                        # pull region-a partial back into PSUM via identity
                        # matmul
                        nc.tensor.matmul(ops[:, :w], ident_sb[:], sl,
                                         start=True, stop=False)
                    for ri in range(R):
                        nc.tensor.matmul(ops[:, :w], wrel_sb[:, ri, :],
                                         accTs[ri][:, :w],
                                         start=False,
                                         stop=(ri == R - 1))
                    if reg == 0:
                        nc.scalar.activation(sl, ops[:, :w], Act.Identity,
                                             bias=bias_sb[:, 2:3])
                    else:
                        nc.scalar.activation(sl, ops[:, :w], Act.Copy)
                        if nxt is not None:
                            transpose_store(sl, t * TW, w, ylo_d, yhi_d)
                            if t == RA // TW - 1:
                                all_gather(ylo_d, nxt[0])
                            elif t == NT - 1:
                                all_gather(yhi_d, nxt[1])
                        else:
                            # last layer: fuse the output MLP per finished
                            # tile so it overlaps the remaining aggregation
                            ps_o = psp.tile([128, TW], f32, tag="out", bufs=2)
                            nc.tensor.matmul(ps_o[:, :w], wout1_sb[:], sl,
                                             start=True, stop=True)
                            z1 = wk.tile([128, TW], f16, tag="x0c")
                            nc.scalar.activation(z1[:, :w], ps_o[:, :w],
                                                 Act.Prelu,
                                                 bias=bias_sb[:, 3:4],
                                                 alpha=SLOPE)
                            ps2 = psp.tile([2, TW], f32, tag="out2")
                            nc.tensor.matmul(ps2[:, :w], wout2_sb[:],
                                             z1[:, :w], start=True, stop=True)
                            nc.scalar.activation(outT[:, t * TW:t * TW + w],
                                                 ps2[:, :w], Act.Identity,
                                                 bias=bout2_sb[:, 0:1])
                    accTs.clear()

                blk_i = 0
                cur = None  # (region, t)
                for (reg, s0, ntok, soff, ccols) in chunks:
                    nb = ntok // 128
                    g = wk.tile([128, CHUNK // 128, D], f16, tag="g", bufs=20)
                    st = wk.tile([128, SMAXC], f16, tag="st", bufs=20)
                    nc.sync.dma_start(out=st[:, :ccols],
                                      in_=strips_d[:, soff:soff + ccols])
                    q = qctr[0] % NQUEUES
                    nc.gpsimd.dma_gather(
                        out_ap=g[:, :nb, :],
                        in_ap=xf_base[reg][:],
                        idxs_ap=gidx_sb[:, s0 // 16:(s0 + ntok) // 16],
                        num_idxs=ntok,
                        num_idxs_reg=ntok,
                        elem_size=D,
                        queue_num=q,
                    )
                    qctr[0] += 1
                    for j in range(nb):
                        breg, bt, br, first, last, lo, span, so = sched[blk_i]
                        assert breg == reg
                        if cur is None:
                            cur = (breg, bt)
                        elif cur != (breg, bt):
                            finish_tile(*cur)
                            cur = (breg, bt)
                        w = _tile_w(bt)
                        if first:
                            agg[br] = psp.tile([128, TW], f32, tag="agg",
                                               name=f"agg{br}", bufs=4)
                            # zero content: narrow matmuls leave untouched
                            # columns' CONTENT stale (start=True only clears
                            # has_written); the transform reads [0, w)
                            nc.vector.memset(agg[br][:, :w], 0.0)
                        sloc = so - soff
                        nc.tensor.matmul(agg[br][:, lo:lo + span], g[:, j, :],
                                         st[:, sloc:sloc + span],
                                         start=first, stop=last)
                        if last:
                            acc = wk.tile([128, TW], f16, tag="accT", bufs=12)
                            nc.scalar.activation(acc[:, :w], agg[br][:, :w],
                                                 Act.Copy)
                            accTs[br] = acc
                        blk_i += 1
                finish_tile(*cur)
                assert blk_i == len(sched)
                return yT

            outT = big.tile([2, L], f32, tag="outT")
            y1T = rgcn_layer(xf_a[0], xf_b[0], x1T, y_lo[1], y_hi[1],
                             nxt=(xf_a[1], xf_b[1]))
            rgcn_layer(xf_a[1], xf_b[1], y1T, None, None, nxt=None)
            nc.sync.dma_start(out=out_d[:], in_=outT[:])

    nc.compile()
    return nc


def kernel(des, tweet, num_prop, cat_prop, edge_index, edge_type,
           W_des, b_des, W_in, b_in, W_rel, W_root, b_rgcn,
           W_out1, b_out1, W_out2, b_out2):
    import time
    from concourse.bass_utils import run_bass_kernel_spmd

    des = np.asarray(des, np.float32)
    gidx_w, strips, sched, chunks, TTOT, TOTCOL, SMAXC = _prep_edges(
        np.asarray(edge_index), np.asarray(edge_type))

    t0 = time.time()
    nc = _build(sched, chunks, TTOT, TOTCOL, SMAXC)
    t1 = time.time()

    des_pad = np.zeros((NPAD, DDES), np.float16)
    des_pad[:N] = des.astype(np.float16)
    bias = np.stack([np.asarray(b_des, np.float32),
                     np.asarray(b_in, np.float32),
                     np.asarray(b_rgcn, np.float32),
                     np.asarray(b_out1, np.float32)], axis=1)  # [128,4]
    common = {
        "wdes": np.asarray(W_des, np.float16),
        "win": np.asarray(W_in, np.float16),
        "wroot": np.asarray(W_root, np.float16),
        "wrel": np.asarray(W_rel, np.float16),
        "wout1": np.asarray(W_out1, np.float16),
        "wout2": np.asarray(W_out2, np.float16),
        "bias": bias,
        "bout2": np.asarray(b_out2, np.float32).reshape(2, 1),
    }
    in_maps = []
    for m in range(M):
        in_maps.append({
            "desT": np.ascontiguousarray(des_pad[m * L:(m + 1) * L].T),
            "gidx": gidx_w[m],
            "strips": strips[m],
            **common,
        })

    trace = bool(_LAST.get("trace"))
    res = run_bass_kernel_spmd(nc, in_maps, list(range(M)), trace=trace)
    t2 = time.time()
    _LAST["build_s"] = t1 - t0
    _LAST["run_s"] = t2 - t1
    _LAST["exec_ns"] = res.exec_time_ns
    _LAST["ttot"] = TTOT

    out = np.concatenate([res.results[m]["out"].T for m in range(M)], axis=0)
    return np.ascontiguousarray(out[:N])
